# revision 25
# baseline (speedup 1.0000x reference)
"""Trainium2 Bass kernel for GQA attention layer (B=1, S=2048, D=4096,
NH=32, NKV=8, HD=128) with RoPE + KV cache append.

Sharding: tensor-parallel over heads across 8 cores. Each core owns 4 Q
heads and 1 KV head; x is replicated; the wo all-reduce is done on the
host (sum of per-core partials), cache outputs are concatenated on host.

Device-side layout trick: scores are computed TRANSPOSED ([t, s_q]) so
that the softmax'd tiles feed the PV matmul directly as the moving
operand (no per-tile transposes of probabilities). Softmax is max-free
(scores are O(10), exp is safe in fp32); the denominator is accumulated
with DVE adds and reduced over partitions with a ones-matmul.

All DRAM inputs are pre-transposed on the host so every DMA is natural
(fp32 has no DMA-transpose path on trn2). Matmul operands are fp32r
(full-rate fp32 mode, needs N>=256); tensors feeding matmuls are typed
fp32r end-to-end since the walrus verifier requires the producing
instruction to round to fp32r.
"""

import sys

sys.path.insert(0, "/opt/trn_rl_repo")

from contextlib import ExitStack

import numpy as np

import concourse.bacc as bacc
import concourse.bass as bass
import concourse.mybir as mybir
import concourse.tile as tile
from concourse import bass_utils, masks

F32 = mybir.dt.float32
F32R = mybir.dt.float32r
PSUM = bass.MemorySpace.PSUM
EXP = mybir.ActivationFunctionType.Exp

B, S, D = 1, 2048, 4096
NH, NKV, HD = 32, 8, 128
NCORES = 8
QH = NH // NCORES          # 4 q heads per core
OQ = QH * HD               # 512
NEG = -1e9
ST = S // 128              # 16 s-tiles
DT = D // 128              # 32 d-tiles
CH = 512                   # s_q chunk width (one PSUM bank of fp32)
NCH = S // CH              # 4 chunks
FCH = D // 512             # 8 output-feature chunks

_cache = {}


def _build(variant: str, use_f32r: bool):
    """variant: 'causal' | 'nomask' | 'generic'"""
    nc = bacc.Bacc("TRN2", target_bir_lowering=False, debug=False,
                   num_devices=NCORES)
    MMDT = F32R if use_f32r else F32

    xT = nc.dram_tensor("xT", [D, S], MMDT, kind="ExternalInput").ap()
    wqT = nc.dram_tensor("wqT", [D, OQ], MMDT, kind="ExternalInput").ap()
    wkvT = nc.dram_tensor("wkvT", [D, 2 * HD], MMDT, kind="ExternalInput").ap()
    woT = nc.dram_tensor("woT", [OQ, D], MMDT, kind="ExternalInput").ap()
    cosv = nc.dram_tensor("cosv", [S, HD // 2], F32, kind="ExternalInput").ap()
    sinv = nc.dram_tensor("sinv", [S, HD // 2], F32, kind="ExternalInput").ap()
    if variant == "generic":
        maskT = nc.dram_tensor("maskT", [S, S], F32, kind="ExternalInput").ap()
    partial = nc.dram_tensor("partial", [S, D], F32, kind="ExternalOutput").ap()
    kout = nc.dram_tensor("kout", [S, HD], F32, kind="ExternalOutput").ap()
    vout = nc.dram_tensor("vout", [S, HD], F32, kind="ExternalOutput").ap()
    qT_d = nc.dram_tensor("qT_scratch", [OQ, S], MMDT, kind="Internal").ap()

    xT_r = xT.rearrange("(a p) s -> p a s", p=128)
    wqT_r = wqT.rearrange("(a p) o -> p a o", p=128)
    wkvT_r = wkvT.rearrange("(a p) o -> p a o", p=128)
    woT_r = woT.rearrange("(a p) f -> p a f", p=128)

    with tile.TileContext(nc) as tc, ExitStack() as octx:
        pers = octx.enter_context(tc.tile_pool(name="pers", bufs=1))
        identity = pers.tile([128, 128], F32)
        masks.make_identity(nc, identity[:])
        ones_mat = pers.tile([128, 128], F32)
        nc.gpsimd.memset(ones_mat[:], 1.0)
        cs_sb = pers.tile([128, ST, 2, HD // 2], F32)
        nc.sync.dma_start(cs_sb[:, :, 0, :], cosv.rearrange("(a p) j -> p a j", p=128))
        nc.sync.dma_start(cs_sb[:, :, 1, :], sinv.rearrange("(a p) j -> p a j", p=128))
        kT_sb = pers.tile([128, S], MMDT)       # [hd, t]
        v_sb = pers.tile([128, ST, HD], F32)    # exact v for cache output
        v_r = pers.tile([128, ST, HD], MMDT)    # rounded v for PV matmul

        # ---------------- stage 1: QKV projections + rope -------------
        with tc.tile_pool(name="w1", bufs=1) as wpool, \
             tc.tile_pool(name="xs", bufs=2) as xpool, \
             tc.tile_pool(name="rp", bufs=3) as rpool, \
             tc.tile_pool(name="st1", bufs=3) as spool, \
             tc.tile_pool(name="ps_q", bufs=2, space=PSUM) as ps_q, \
             tc.tile_pool(name="ps_kv", bufs=2, space=PSUM) as ps_kv, \
             tc.tile_pool(name="ps_t", bufs=2, space=PSUM) as ps_t:
            wq_sb = wpool.tile([128, DT, OQ], MMDT)
            nc.sync.dma_start(wq_sb[:], wqT_r)
            wkv_sb = wpool.tile([128, DT, 2 * HD], MMDT)
            nc.sync.dma_start(wkv_sb[:], wkvT_r)

            for st in range(ST):
                x_sb = xpool.tile([128, DT, 128], MMDT, tag="x")
                nc.sync.dma_start(x_sb[:], xT_r[:, :, st * 128:(st + 1) * 128])

                q_ps = ps_q.tile([128, OQ], F32, tag="q")
                kv_ps = ps_kv.tile([128, 2 * HD], F32, tag="kv")
                for dt_i in range(DT):
                    nc.tensor.matmul(q_ps[:], x_sb[:, dt_i, :],
                                     wq_sb[:, dt_i, :],
                                     start=(dt_i == 0), stop=(dt_i == DT - 1))
                for dt_i in range(DT):
                    nc.tensor.matmul(kv_ps[:], x_sb[:, dt_i, :],
                                     wkv_sb[:, dt_i, :],
                                     start=(dt_i == 0), stop=(dt_i == DT - 1))

                # rope on natural-layout tiles (pairs along free dim)
                cos_t = cs_sb[:, st, 0, :]
                sin_t = cs_sb[:, st, 1, :]
                qr = rpool.tile([128, OQ], F32, tag="qr")
                kr = rpool.tile([128, HD], F32, tag="kr")
                tmp = rpool.tile([128, HD], F32, tag="tmp")

                def rope(dst, src):
                    xr = src[:, 0:HD:2]
                    xi = src[:, 1:HD:2]
                    orr = dst[:, 0:HD:2]
                    oi = dst[:, 1:HD:2]
                    nc.vector.tensor_mul(orr, xr, cos_t)
                    nc.vector.tensor_mul(tmp[:, 0:HD // 2], xi, sin_t)
                    nc.vector.tensor_sub(orr, orr, tmp[:, 0:HD // 2])
                    nc.vector.tensor_mul(oi, xr, sin_t)
                    nc.vector.tensor_mul(tmp[:, HD // 2:HD], xi, cos_t)
                    nc.vector.tensor_add(oi, oi, tmp[:, HD // 2:HD])

                for h in range(QH):
                    rope(qr[:, h * HD:(h + 1) * HD], q_ps[:, h * HD:(h + 1) * HD])
                rope(kr[:], kv_ps[:, 0:HD])
                nc.scalar.copy(v_sb[:, st, :], kv_ps[:, HD:2 * HD])
                nc.scalar.copy(v_r[:, st, :], kv_ps[:, HD:2 * HD])

                nc.sync.dma_start(kout[st * 128:(st + 1) * 128, :], kr[:])
                nc.sync.dma_start(vout[st * 128:(st + 1) * 128, :], v_sb[:, st, :])

                # transpose q -> DRAM scratch, k -> resident kT
                for h in range(QH):
                    pt = ps_t.tile([128, 128], F32, tag="pt")
                    nc.tensor.transpose(pt[:], qr[:, h * HD:(h + 1) * HD], identity[:])
                    sg = spool.tile([128, 128], MMDT, tag="sg")
                    nc.scalar.copy(sg[:], pt[:])
                    nc.sync.dma_start(
                        qT_d[h * 128:(h + 1) * 128, st * 128:(st + 1) * 128], sg[:])
                pt = ps_t.tile([128, 128], F32, tag="pt")
                nc.tensor.transpose(pt[:], kr[:], identity[:])
                nc.scalar.copy(kT_sb[:, st * 128:(st + 1) * 128], pt[:])

        # ---------------- stage 2: attention --------------------------
        with tc.tile_pool(name="wo", bufs=1) as wopool, \
             tc.tile_pool(name="s2", bufs=1) as s2pool:
            wo_sb = wopool.tile([128, QH, D], MMDT)
            nc.sync.dma_start(wo_sb[:], woT_r)
            outT_sb = s2pool.tile([128, QH, S], MMDT)   # [hd, h, s_q]

            with tc.tile_pool(name="qh", bufs=2) as qhpool, \
                 tc.tile_pool(name="dn", bufs=2) as dnpool, \
                 tc.tile_pool(name="ex", bufs=4) as epool, \
                 tc.tile_pool(name="mk", bufs=4) as mkpool, \
                 tc.tile_pool(name="rcp", bufs=2) as rcpool, \
                 tc.tile_pool(name="ps_o", bufs=1, space=PSUM) as ps_o, \
                 tc.tile_pool(name="ps_s", bufs=3, space=PSUM) as ps_s:
                if variant == "causal":
                    caus = s2pool.tile([128, 4, CH], F32)
                    for o in range(4):
                        nc.gpsimd.memset(caus[:, o, :], 0.0)
                        # keep 0 where t <= s_q i.e. (j - p - off) >= 0
                        nc.gpsimd.affine_select(
                            out=caus[:, o, :], in_=caus[:, o, :],
                            compare_op=mybir.AluOpType.is_ge, fill=NEG,
                            base=-o * 128, pattern=[[1, CH]],
                            channel_multiplier=-1)

                for h in range(QH):
                    qh_sb = qhpool.tile([128, S], MMDT, tag="qh")
                    nc.sync.dma_start(qh_sb[:], qT_d[h * 128:(h + 1) * 128, :])
                    den = dnpool.tile([128, S], F32, tag="den")
                    nc.gpsimd.memset(den[:], 0.0)
                    o_ps = [ps_o.tile([128, CH], F32, tag=f"o{c}",
                                      name=f"o_ps{c}")
                            for c in range(NCH)]

                    for kt in range(ST):
                        c_lo = kt // 4 if variant == "causal" else 0
                        for c in range(c_lo, NCH):
                            s_ps = ps_s.tile([128, CH], F32, tag="s")
                            nc.tensor.matmul(
                                s_ps[:], kT_sb[:, kt * 128:(kt + 1) * 128],
                                qh_sb[:, c * CH:(c + 1) * CH],
                                start=True, stop=True)
                            e_t = epool.tile([128, CH], MMDT, tag="e")
                            e_f = e_t[:].bitcast(F32)
                            if variant == "causal" and c == kt // 4:
                                sc_t = mkpool.tile([128, CH], F32, tag="sc")
                                nc.vector.tensor_add(sc_t[:], s_ps[:],
                                                     caus[:, kt % 4, :])
                                nc.scalar.activation(e_t[:], sc_t[:], EXP)
                            elif variant == "generic":
                                m_t = mkpool.tile([128, CH], F32, tag="m")
                                nc.sync.dma_start(
                                    m_t[:], maskT[kt * 128:(kt + 1) * 128,
                                                  c * CH:(c + 1) * CH])
                                sc_t = mkpool.tile([128, CH], F32, tag="sc")
                                nc.vector.tensor_add(sc_t[:], s_ps[:], m_t[:])
                                nc.scalar.activation(e_t[:], sc_t[:], EXP)
                            else:
                                nc.scalar.activation(e_t[:], s_ps[:], EXP)
                            nc.vector.tensor_add(den[:, c * CH:(c + 1) * CH],
                                                 den[:, c * CH:(c + 1) * CH],
                                                 e_f)
                            last_kt = min(4 * c + 3, ST - 1) \
                                if variant == "causal" else ST - 1
                            nc.tensor.matmul(o_ps[c][:], v_r[:, kt, :],
                                             e_t[:],
                                             start=(kt == 0),
                                             stop=(kt == last_kt))

                    # normalize: ones-matmul = partition-sum broadcast to all
                    # 128 rows in one shot, then elementwise divide.
                    for c in range(NCH):
                        d_ps = ps_s.tile([128, CH], F32, tag="s")
                        nc.tensor.matmul(d_ps[:], ones_mat[:],
                                         den[:, c * CH:(c + 1) * CH],
                                         start=True, stop=True)
                        b_sb = rcpool.tile([128, CH], F32, tag="b_sb")
                        nc.vector.reciprocal(b_sb[:], d_ps[:])
                        nc.vector.tensor_mul(
                            outT_sb[:, h, c * CH:(c + 1) * CH],
                            o_ps[c][:], b_sb[:])

            # ---------------- stage 3: output projection --------------
            with tc.tile_pool(name="ps3", bufs=6, space=PSUM) as ps3, \
                 tc.tile_pool(name="st3", bufs=2) as st3:
                for stile in range(ST):
                    p_sb = st3.tile([128, FCH, 512], F32, tag="p_sb")
                    for fc in range(FCH):
                        p_ps = ps3.tile([128, 512], F32, tag="p")
                        for ct in range(QH):
                            nc.tensor.matmul(
                                p_ps[:],
                                outT_sb[:, ct, stile * 128:(stile + 1) * 128],
                                wo_sb[:, ct, fc * 512:(fc + 1) * 512],
                                start=(ct == 0), stop=(ct == QH - 1))
                        if fc % 2 == 0:
                            nc.vector.tensor_copy(p_sb[:, fc, :], p_ps[:])
                        else:
                            nc.scalar.copy(p_sb[:, fc, :], p_ps[:])
                    nc.sync.dma_start(
                        partial[stile * 128:(stile + 1) * 128, :], p_sb[:])

    nc.compile()
    return nc


def _get_nc(variant: str, use_f32r: bool):
    key = (variant, use_f32r)
    if key not in _cache:
        _cache[key] = _build(variant, use_f32r)
    return _cache[key]


def _detect_variant(mask):
    if not mask.any():
        return "nomask"
    row = np.arange(S, dtype=np.int64)[:, None]
    col = np.arange(S, dtype=np.int64)[None, :]
    causal = np.where(col <= row, np.float32(0.0), np.float32(NEG))
    if np.array_equal(mask, causal):
        return "causal"
    return "generic"


def kernel(x, freqs_cos, freqs_sin, mask, input_indexes, cache_k, cache_v,
           wq, wk, wv, wo, use_f32r=True, trace=False):
    x = np.ascontiguousarray(np.asarray(x, np.float32))
    mask2d = np.asarray(mask, np.float32).reshape(S, S)
    variant = _detect_variant(mask2d)
    nc = _get_nc(variant, use_f32r)

    xT = np.ascontiguousarray(x.reshape(S, D).T)
    scale = np.float32(1.0 / np.sqrt(HD))
    wqT = np.ascontiguousarray(np.asarray(wq, np.float32).T) * scale  # [D, NH*HD]
    wkT = np.ascontiguousarray(np.asarray(wk, np.float32).T)          # [D, NKV*HD]
    wvT = np.ascontiguousarray(np.asarray(wv, np.float32).T)
    woT = np.ascontiguousarray(np.asarray(wo, np.float32).T)          # [NH*HD, D]
    cos = np.ascontiguousarray(np.asarray(freqs_cos, np.float32))
    sin = np.ascontiguousarray(np.asarray(freqs_sin, np.float32))
    maskT = np.ascontiguousarray(mask2d.T) if variant == "generic" else None

    in_maps = []
    for m in range(NCORES):
        im = {
            "xT": xT,
            "wqT": np.ascontiguousarray(wqT[:, m * OQ:(m + 1) * OQ]),
            "wkvT": np.ascontiguousarray(np.concatenate(
                [wkT[:, m * HD:(m + 1) * HD], wvT[:, m * HD:(m + 1) * HD]],
                axis=1)),
            "woT": np.ascontiguousarray(woT[m * OQ:(m + 1) * OQ, :]),
            "cosv": cos,
            "sinv": sin,
        }
        if variant == "generic":
            im["maskT"] = maskT
        in_maps.append(im)

    res = bass_utils.run_bass_kernel_spmd(
        nc, in_maps, core_ids=list(range(NCORES)), trace=trace)
    results = res.results

    out = np.zeros((S, D), np.float64)
    for m in range(NCORES):
        out += results[m]["partial"].astype(np.float64)
    out = out.astype(np.float32).reshape(B, S, D)

    idx = np.asarray(input_indexes).astype(np.int64)
    k_new = np.stack([results[m]["kout"] for m in range(NCORES)], axis=1)
    v_new = np.stack([results[m]["vout"] for m in range(NCORES)], axis=1)
    ck = np.array(np.asarray(cache_k, np.float32), copy=True).reshape(B, S, NKV, HD)
    cv = np.array(np.asarray(cache_v, np.float32), copy=True).reshape(B, S, NKV, HD)
    ck[0, idx] = k_new
    cv[0, idx] = v_new

    if trace:
        kernel._last_result = res
    return (out, ck, cv)


kernel._last_result = None
